# revision 20
# baseline (speedup 1.0000x reference)
"""Trainium2 Bass kernel for nn_AttentionCell (Bahdanau attention + LSTM cell step).

Data-parallel over batch across 8 NeuronCores: each core gets B/8 = 64 batch
rows (all weights replicated) and computes its slice of (h_new, c_new, alpha)
independently — no collectives.

Per-core pipeline (shard b = 64, BT = b*T = 16384, all matmuls bf16 / f32 acc):
  phase A: hp^T = Wh^T prev_h^T + bh           (PE, tiny)
  phase B: proj^T tiles [128h, 512bt] = Wi^T @ batch_H^T   (PE)
           tanh(proj^T + hp^T) via ACT with per-partition bias   (ACT -> bf16)
           e = Ws^T tanh-tiles (M=1 matmul, accumulated over h)  (PE)
  phase C: softmax over t (free dim) -> alpha [64, 256]; build the
           block-diagonal alpha operand via PE transpose + strided DVE copy
  phase D: context[64, 512] = alpha_blockdiag^T @ batch_H (natural layout)
           one 128-matmul PSUM accumulation                       (PE)
  phase E: z = [context|onehots|prev_h|1] @ [K;R;b] -> LSTM gates -> h, c

batch_H is passed from the host in BOTH layouts (transposed for phase B,
natural for phase D), pre-cast to bf16, so no on-device transposition of the
big tensor is needed.
"""

import numpy as np
import ml_dtypes

import concourse.bass as bass
import concourse.mybir as mybir
import concourse.tile as tile
from concourse import bacc
from concourse.bass_utils import run_bass_kernel_spmd
from concourse.masks import make_identity
from concourse.tile_rust import add_dep_helper

bf16 = ml_dtypes.bfloat16
F32 = mybir.dt.float32
BF16 = mybir.dt.bfloat16
AF = mybir.ActivationFunctionType
ALU = mybir.AluOpType

NCORES = 8
B, T, C, H, NCLS = 512, 256, 512, 512, 96
P = B // NCORES          # 64 batch rows per core
BT = P * T               # 16384 flattened (b, t) rows per core
KX = C + NCLS + H + 1    # 1121 stacked contraction rows for the LSTM matmul
KXP = 1152               # padded to 9*128
NJ = BT // 512           # 32 bt-chunks of 512
NKT = BT // 128          # 128 bt-chunks of 128

_CACHED = None


def _build():
    nc = bacc.Bacc("TRN2", target_bir_lowering=False, debug=False,
                   num_devices=NCORES)

    bhT = nc.dram_tensor("bhT", [C, BT], BF16, kind="ExternalInput")
    bhN = nc.dram_tensor("bhN", [BT // 4, 4 * C], BF16, kind="ExternalInput")
    wi = nc.dram_tensor("wi", [128, 4 * H], BF16, kind="ExternalInput")
    whE = nc.dram_tensor("whE", [128, 5 * H], BF16, kind="ExternalInput")
    phTe = nc.dram_tensor("phTe", [128, 5 * P], BF16, kind="ExternalInput")
    ws = nc.dram_tensor("ws", [128, 4], BF16, kind="ExternalInput")
    xrt = nc.dram_tensor("xrt", [128, 5 * P], BF16, kind="ExternalInput")
    krE = nc.dram_tensor("krE", [KXP, 4 * H], BF16, kind="ExternalInput")
    pc = nc.dram_tensor("pc", [P, H], F32, kind="ExternalInput")

    e_tmp = nc.dram_tensor("e_tmp", [BT], F32)  # internal staging for reshape
    h_out = nc.dram_tensor("h_out", [P, H], F32, kind="ExternalOutput")
    c_out = nc.dram_tensor("c_out", [P, H], F32, kind="ExternalOutput")
    alpha_out = nc.dram_tensor("alpha_out", [P, T], F32, kind="ExternalOutput")

    with tile.TileContext(nc) as tc:
        with (
            tc.tile_pool(name="const", bufs=1) as cpool,
            tc.tile_pool(name="work", bufs=1) as wpool,
            tc.tile_pool(name="bht", bufs=3) as bht_pool,
            tc.tile_pool(name="tanh", bufs=8) as tanh_pool,
            tc.tile_pool(name="parg", bufs=3) as parg_pool,
            tc.tile_pool(name="bhn", bufs=13) as bhn_pool,
            tc.tile_pool(name="kr", bufs=4) as kr_pool,
        ):
            # ---- constants / weights into SBUF ----
            wi_sb = cpool.tile([128, 4 * H], BF16)           # [p, (kc h)]
            nc.sync.dma_start(wi_sb[:], wi[:])
            ws_sb = cpool.tile([128, 4], BF16)               # [p, kc]
            nc.sync.dma_start(ws_sb[:], ws[:])
            ident_f = cpool.tile([128, 128], F32)
            make_identity(nc, ident_f[:])
            ident_b = cpool.tile([128, 128], BF16)
            nc.vector.tensor_copy(ident_b[:], ident_f[:])

            whE_sb = cpool.tile([128, 5 * H], BF16)
            nc.gpsimd.dma_start(whE_sb[:], whE[:])
            phTe_sb = cpool.tile([128, 5 * P], BF16)
            nc.gpsimd.dma_start(phTe_sb[:], phTe[:])

            # block-diagonal alpha operand [p, (kt m)] for phase D
            bd_sb = wpool.tile([128, NKT * P], BF16)
            nc.vector.memset(bd_sb[:], 0.0)

            # prefetch phase-D rhs tiles early on the SWDGE queue (4 kt per DMA)
            bhn_t = []
            for g in range(NKT // 4):
                bt_g = bhn_pool.tile([128, 4 * C], BF16, tag="bhn", name=f"bhn_t{g}")
                nc.gpsimd.dma_start(bt_g[:], bhN[g * 128:(g + 1) * 128, :])
                bhn_t.append(bt_g)

            # ---- phase A: hp^T [128h x 4, 64b] (+bh via ones row) ----
            hpT_sb = wpool.tile([128, 4 * P], F32)
            with tc.tile_pool(name="psA", bufs=2, space="PSUM") as psA:
                for hc in range(4):
                    hp_ps = psA.tile([128, P], F32)
                    for kk in range(5):
                        nc.tensor.matmul(
                            hp_ps[:],
                            whE_sb[:, kk * H + hc * 128:kk * H + (hc + 1) * 128],
                            phTe_sb[:, kk * P:(kk + 1) * P],
                            start=(kk == 0), stop=(kk == 4),
                        )
                    nc.vector.tensor_copy(hpT_sb[:, hc * P:(hc + 1) * P], hp_ps[:])

            # ---- phase B: proj -> tanh -> e ----
            # e chunks land at partition base 32*(j%4), column slot j//4
            # (DVE dest partition bases must be 32-aligned)
            e_stage = wpool.tile([128, (NJ // 4) * 512], F32)
            with (
                tc.tile_pool(name="psB", bufs=6, space="PSUM") as psB,
                tc.tile_pool(name="psE", bufs=2, space="PSUM") as psE,
            ):
                # e-matmuls are software-pipelined 2 (j,hc)-groups behind the
                # proj/tanh that feed them, so the in-order PE queue never
                # head-of-line-blocks on an unfinished ACT pair.
                pending = []
                last_proj = [None]

                def flush_e():
                    # emit all 4 e-matmuls of one j back-to-back: the PE pays
                    # its 128<->32 tile-size reconfig penalty twice per j
                    # instead of on every e-matmul
                    for _ in range(4):
                        e_ps_f, hc_f, tanh_f, j_f = pending.pop(0)
                        mm = nc.tensor.matmul(
                            e_ps_f[:],
                            ws_sb[:, hc_f:hc_f + 1],
                            tanh_f[:],
                            start=(hc_f == 0), stop=(hc_f == 3),
                        )
                        if last_proj[0] is not None:
                            add_dep_helper(mm.ins, last_proj[0].ins, False,
                                           "software-pipeline e-mm behind proj")
                    g, s = (j_f % 4) * 32, j_f // 4
                    nc.scalar.copy(
                        e_stage[g:g + 1, s * 512:(s + 1) * 512], e_ps_f[:])

                bht_t = None
                for j in range(NJ):
                    if j % 4 == 0:
                        bht_t = [bht_pool.tile([128, 4 * 512], BF16, tag=f"bht{kc}",
                                               name=f"bht_t{j}_{kc}")
                                 for kc in range(4)]
                        for kc in range(4):
                            if j == 0:
                                # split the first group per-j so early slices
                                # land ASAP and the PE can start early
                                for jj2 in range(4):
                                    nc.sync.dma_start(
                                        bht_t[kc][:, jj2 * 512:(jj2 + 1) * 512],
                                        bhT[kc * 128:(kc + 1) * 128,
                                            jj2 * 512:(jj2 + 1) * 512],
                                    )
                            elif j == 4:
                                for jj2 in range(2):
                                    nc.sync.dma_start(
                                        bht_t[kc][:, jj2 * 1024:(jj2 + 1) * 1024],
                                        bhT[kc * 128:(kc + 1) * 128,
                                            (4 + jj2 * 2) * 512:(6 + jj2 * 2) * 512],
                                    )
                            else:
                                nc.sync.dma_start(
                                    bht_t[kc][:],
                                    bhT[kc * 128:(kc + 1) * 128,
                                        j * 512:(j + 4) * 512],
                                )
                    jj = j % 4
                    e_ps = psE.tile([1, 512], F32)
                    for hc in range(4):
                        proj_ps = psB.tile([128, 512], F32)
                        for kc in range(4):
                            last_proj[0] = nc.tensor.matmul(
                                proj_ps[:],
                                wi_sb[:, kc * H + hc * 128:kc * H + (hc + 1) * 128],
                                bht_t[kc][:, jj * 512:(jj + 1) * 512],
                                start=(kc == 0), stop=(kc == 3),
                            )
                        # hp bias-add on DVE (PSUM -> SBUF f32 staging), then a
                        # single whole-tile tanh on ACT — keeps every engine
                        # under the PE group pace
                        parg_t = parg_pool.tile([128, 512], F32)
                        for half in range(2):
                            b_idx = 2 * j + half
                            nc.vector.tensor_scalar_add(
                                parg_t[:, half * 256:(half + 1) * 256],
                                proj_ps[:, half * 256:(half + 1) * 256],
                                hpT_sb[:, hc * P + b_idx:hc * P + b_idx + 1],
                            )
                        tanh_t = tanh_pool.tile([128, 512], BF16)
                        nc.scalar.activation(tanh_t[:], parg_t[:], AF.Tanh)
                        pending.append((e_ps, hc, tanh_t, j))
                        if len(pending) >= 8:
                            flush_e()
                while pending:
                    flush_e()

            # ---- phase C: softmax over t, alpha block-diagonal build ----
            # DRAM round-trip to reshape [(j) chunks] -> [b, t]; the read DMA
            # gets an explicit dep on the write DMA (DRAM is not Tile-tracked)
            ew = nc.sync.dma_start(
                e_tmp[:].rearrange("(s g n) -> g s n", s=NJ // 4, g=4),
                e_stage[0:128:32, :].rearrange("g (s n) -> g s n", s=NJ // 4),
            )
            e_bt = wpool.tile([P, T], F32)
            er = nc.sync.dma_start(e_bt[:], e_tmp[:].rearrange("(b t) -> b t", b=P))
            add_dep_helper(er.ins, ew.ins, True, "e_tmp write -> read ordering")

            mx_neg = wpool.tile([P, 1], F32)
            nc.vector.tensor_reduce(mx_neg[:], e_bt[:], mybir.AxisListType.X,
                                    ALU.max, negate=True)
            ex = wpool.tile([P, T], F32)
            nc.scalar.activation(ex[:], e_bt[:], AF.Exp, bias=mx_neg[:])
            ssum = wpool.tile([P, 1], F32)
            nc.vector.tensor_reduce(ssum[:], ex[:], mybir.AxisListType.X, ALU.add)
            rsum = wpool.tile([P, 1], F32)
            nc.vector.reciprocal(rsum[:], ssum[:])
            alpha_sb = wpool.tile([P, T], F32)
            nc.vector.tensor_scalar_mul(alpha_sb[:], ex[:], rsum[:])
            nc.sync.dma_start(alpha_out[:], alpha_sb[:])

            with tc.tile_pool(name="psC", bufs=2, space="PSUM") as psC:
                for i in range(2):
                    at_ps = psC.tile([128, P], F32)
                    nc.tensor.transpose(
                        at_ps[:], alpha_sb[0:P, i * 128:(i + 1) * 128],
                        ident_f[0:P, 0:P],
                    )
                    # scatter columns into the block-diagonal layout:
                    # dest free offset = 129*b + 64*i, stride 129, count 64
                    nc.vector.tensor_copy(
                        bd_sb[:, P * i:P * i + 129 * (P - 1) + 1:129], at_ps[:]
                    )

            # ---- phase D: context [64, 512] ----
            ctx_bf = wpool.tile([P, H], BF16)
            with tc.tile_pool(name="psD", bufs=1, space="PSUM") as psD:
                ctx_ps = psD.tile([P, H], F32)
                for kt in range(NKT):
                    g, i = kt // 4, kt % 4
                    nc.tensor.matmul(
                        ctx_ps[:],
                        bd_sb[:, kt * P:(kt + 1) * P],
                        bhn_t[g][:, i * C:(i + 1) * C],
                        start=(kt == 0), stop=(kt == NKT - 1),
                    )
                nc.vector.tensor_copy(ctx_bf[:], ctx_ps[:])

            # ---- phase E: LSTM cell ----
            xstack = wpool.tile([128, 9 * P], BF16)
            nc.sync.dma_start(xstack[:, 4 * P:9 * P], xrt[:])
            with tc.tile_pool(name="psT", bufs=2, space="PSUM") as psT:
                for q in range(4):
                    ct_ps = psT.tile([128, P], BF16)
                    nc.tensor.transpose(
                        ct_ps[:], ctx_bf[0:P, q * 128:(q + 1) * 128],
                        ident_b[0:P, 0:P],
                    )
                    nc.vector.tensor_copy(xstack[:, q * P:(q + 1) * P], ct_ps[:])

            gi = wpool.tile([P, H], F32)
            gf = wpool.tile([P, H], F32)
            gg = wpool.tile([P, H], F32)
            go = wpool.tile([P, H], F32)
            with tc.tile_pool(name="psZ", bufs=1, space="PSUM") as psZ:
                z_ps = psZ.tile([P, 4 * H], F32)
                for kk in range(9):
                    kr_t = kr_pool.tile([128, 4 * H], BF16, tag="kr", name=f"kr_t{kk}")
                    nc.sync.dma_start(kr_t[:], krE[kk * 128:(kk + 1) * 128, :])
                    for nn in range(4):
                        nc.tensor.matmul(
                            z_ps[:, nn * 512:(nn + 1) * 512],
                            xstack[:, kk * P:(kk + 1) * P],
                            kr_t[:, nn * 512:(nn + 1) * 512],
                            start=(kk == 0), stop=(kk == 8),
                        )
                nc.scalar.activation(gi[:], z_ps[:, 0:512], AF.Sigmoid)
                nc.scalar.activation(gf[:], z_ps[:, 512:1024], AF.Sigmoid)
                nc.scalar.activation(gg[:], z_ps[:, 1024:1536], AF.Tanh)
                nc.scalar.activation(go[:], z_ps[:, 1536:2048], AF.Sigmoid)

            pc_sb = wpool.tile([P, H], F32)
            nc.sync.dma_start(pc_sb[:], pc[:])
            t1 = wpool.tile([P, H], F32)
            t2 = wpool.tile([P, H], F32)
            c_sb = wpool.tile([P, H], F32)
            nc.vector.tensor_mul(t1[:], gf[:], pc_sb[:])
            nc.vector.tensor_mul(t2[:], gi[:], gg[:])
            nc.vector.tensor_add(c_sb[:], t1[:], t2[:])
            nc.sync.dma_start(c_out[:], c_sb[:])
            tc_sb = wpool.tile([P, H], F32)
            nc.scalar.activation(tc_sb[:], c_sb[:], AF.Tanh)
            h_sb = wpool.tile([P, H], F32)
            nc.vector.tensor_mul(h_sb[:], go[:], tc_sb[:])
            nc.sync.dma_start(h_out[:], h_sb[:])

    nc.compile()
    return nc


def _get_nc():
    global _CACHED
    if _CACHED is None:
        _CACHED = _build()
    return _CACHED


def _prep_in_maps(inputs):
    batch_H = np.asarray(inputs["batch_H"], np.float32)
    prev_h = np.asarray(inputs["prev_h"], np.float32)
    prev_c = np.asarray(inputs["prev_c"], np.float32)
    onehots = np.asarray(inputs["char_onehots"], np.float32)
    Wi = np.asarray(inputs["Wi"], np.float32)
    Wh = np.asarray(inputs["Wh"], np.float32)
    bh = np.asarray(inputs["bh"], np.float32)
    Ws = np.asarray(inputs["Ws"], np.float32)
    K = np.asarray(inputs["K"], np.float32)
    R = np.asarray(inputs["R"], np.float32)
    b = np.asarray(inputs["b"], np.float32)

    def pack(a, k):  # [(k p), n] -> [p, (k n)]
        n = a.shape[1]
        return np.ascontiguousarray(
            a.reshape(k, 128, n).transpose(1, 0, 2).reshape(128, k * n))

    wi_b = pack(Wi, 4).astype(bf16)
    ws_b = np.ascontiguousarray(Ws.reshape(4, 128).T).astype(bf16)
    whE = np.zeros((640, H), np.float32)
    whE[:H] = Wh
    whE[H] = bh
    whE_b = pack(whE, 5).astype(bf16)
    krE = np.zeros((KXP, 4 * H), np.float32)
    krE[:C + NCLS] = K
    krE[C + NCLS:C + NCLS + H] = R
    krE[C + NCLS + H] = b
    krE_b = krE.astype(bf16)

    in_maps = []
    for i in range(NCORES):
        sl = slice(i * P, (i + 1) * P)
        bh_s = batch_H[sl].reshape(BT, C)
        ph_s = prev_h[sl]                       # [64, 512]
        phTe = np.zeros((640, P), np.float32)
        phTe[:H] = ph_s.T
        phTe[H] = 1.0
        xrt = np.zeros((640, P), np.float32)
        xrt[:NCLS] = onehots[sl].T
        xrt[NCLS:NCLS + H] = ph_s.T
        xrt[NCLS + H] = 1.0
        bhn_p = np.ascontiguousarray(
            bh_s.reshape(32, 4, 128, C).transpose(0, 2, 1, 3).reshape(BT // 4, 4 * C))
        in_maps.append({
            "bhT": np.ascontiguousarray(bh_s.T).astype(bf16),
            "bhN": bhn_p.astype(bf16),
            "wi": wi_b,
            "whE": whE_b,
            "phTe": pack(phTe, 5).astype(bf16),
            "ws": ws_b,
            "xrt": pack(xrt, 5).astype(bf16),
            "krE": krE_b,
            "pc": prev_c[sl],
        })
    return in_maps


def run(inputs, **run_kwargs):
    nc = _get_nc()
    in_maps = _prep_in_maps(inputs)
    res = run_bass_kernel_spmd(nc, in_maps, core_ids=list(range(NCORES)),
                               **run_kwargs)
    h = np.concatenate([res.results[i]["h_out"] for i in range(NCORES)], 0)
    c = np.concatenate([res.results[i]["c_out"] for i in range(NCORES)], 0)
    alpha = np.concatenate([res.results[i]["alpha_out"] for i in range(NCORES)], 0)
    return (h, c, alpha.reshape(B, T, 1)), res


def kernel(**inputs):
    (h, c, alpha), _ = run(inputs)
    return (h, c, alpha)


# revision 21
# speedup vs baseline: 1.0302x; 1.0302x over previous
"""Trainium2 Bass kernel for nn_AttentionCell (Bahdanau attention + LSTM cell step).

Data-parallel over batch across 8 NeuronCores: each core gets B/8 = 64 batch
rows (all weights replicated) and computes its slice of (h_new, c_new, alpha)
independently — no collectives.

Per-core pipeline (shard b = 64, BT = b*T = 16384, all matmuls bf16 / f32 acc):
  phase A: hp^T = Wh^T prev_h^T + bh           (PE, tiny)
  phase B: proj^T tiles [128h, 512bt] = Wi^T @ batch_H^T   (PE)
           tanh(proj^T + hp^T) via ACT with per-partition bias   (ACT -> bf16)
           e = Ws^T tanh-tiles (M=1 matmul, accumulated over h)  (PE)
  phase C: softmax over t (free dim) -> alpha [64, 256]; build the
           block-diagonal alpha operand via PE transpose + strided DVE copy
  phase D: context[64, 512] = alpha_blockdiag^T @ batch_H (natural layout)
           one 128-matmul PSUM accumulation                       (PE)
  phase E: z = [context|onehots|prev_h|1] @ [K;R;b] -> LSTM gates -> h, c

batch_H is passed from the host in BOTH layouts (transposed for phase B,
natural for phase D), pre-cast to bf16, so no on-device transposition of the
big tensor is needed.
"""

import numpy as np
import ml_dtypes

import concourse.bass as bass
import concourse.mybir as mybir
import concourse.tile as tile
from concourse import bacc
from concourse.bass_utils import run_bass_kernel_spmd
from concourse.masks import make_identity
from concourse.tile_rust import add_dep_helper

bf16 = ml_dtypes.bfloat16
F32 = mybir.dt.float32
BF16 = mybir.dt.bfloat16
AF = mybir.ActivationFunctionType
ALU = mybir.AluOpType

NCORES = 8
B, T, C, H, NCLS = 512, 256, 512, 512, 96
P = B // NCORES          # 64 batch rows per core
BT = P * T               # 16384 flattened (b, t) rows per core
KX = C + NCLS + H + 1    # 1121 stacked contraction rows for the LSTM matmul
KXP = 1152               # padded to 9*128
NJ = BT // 512           # 32 bt-chunks of 512
NKT = BT // 128          # 128 bt-chunks of 128

_CACHED = None


def _build():
    nc = bacc.Bacc("TRN2", target_bir_lowering=False, debug=False,
                   num_devices=NCORES)

    bhT = nc.dram_tensor("bhT", [C, BT], BF16, kind="ExternalInput")
    bhN = nc.dram_tensor("bhN", [BT // 4, 4 * C], BF16, kind="ExternalInput")
    wi = nc.dram_tensor("wi", [128, 4 * H], BF16, kind="ExternalInput")
    whE = nc.dram_tensor("whE", [128, 5 * H], BF16, kind="ExternalInput")
    phTe = nc.dram_tensor("phTe", [128, 5 * P], BF16, kind="ExternalInput")
    ws = nc.dram_tensor("ws", [128, 4], BF16, kind="ExternalInput")
    xrt = nc.dram_tensor("xrt", [128, 5 * P], BF16, kind="ExternalInput")
    krE = nc.dram_tensor("krE", [KXP, 4 * H], BF16, kind="ExternalInput")
    pc = nc.dram_tensor("pc", [P, H], F32, kind="ExternalInput")

    e_tmp = nc.dram_tensor("e_tmp", [BT], F32)  # internal staging for reshape
    h_out = nc.dram_tensor("h_out", [P, H], F32, kind="ExternalOutput")
    c_out = nc.dram_tensor("c_out", [P, H], F32, kind="ExternalOutput")
    alpha_out = nc.dram_tensor("alpha_out", [P, T], F32, kind="ExternalOutput")

    with tile.TileContext(nc) as tc:
        with (
            tc.tile_pool(name="const", bufs=1) as cpool,
            tc.tile_pool(name="work", bufs=1) as wpool,
            tc.tile_pool(name="bht", bufs=3) as bht_pool,
            tc.tile_pool(name="tanh", bufs=8) as tanh_pool,
            tc.tile_pool(name="parg", bufs=3) as parg_pool,
            tc.tile_pool(name="bhn", bufs=13) as bhn_pool,
            tc.tile_pool(name="kr", bufs=4) as kr_pool,
        ):
            # ---- constants / weights into SBUF ----
            wi_sb = cpool.tile([128, 4 * H], BF16)           # [p, (kc h)]
            nc.sync.dma_start(wi_sb[:], wi[:])
            ws_sb = cpool.tile([128, 4], BF16)               # [p, kc]
            nc.sync.dma_start(ws_sb[:], ws[:])
            ident_f = cpool.tile([128, 128], F32)
            make_identity(nc, ident_f[:])
            ident_b = cpool.tile([128, 128], BF16)
            nc.vector.tensor_copy(ident_b[:], ident_f[:])

            whE_sb = cpool.tile([128, 5 * H], BF16)
            nc.gpsimd.dma_start(whE_sb[:], whE[:])
            phTe_sb = cpool.tile([128, 5 * P], BF16)
            nc.gpsimd.dma_start(phTe_sb[:], phTe[:])

            # block-diagonal alpha operand [p, (kt m)] for phase D
            bd_sb = wpool.tile([128, NKT * P], BF16)
            nc.vector.memset(bd_sb[:], 0.0)

            # prefetch phase-D rhs tiles early on the SWDGE queue (4 kt per DMA)
            bhn_t = []
            for g in range(NKT // 4):
                bt_g = bhn_pool.tile([128, 4 * C], BF16, tag="bhn", name=f"bhn_t{g}")
                nc.gpsimd.dma_start(bt_g[:], bhN[g * 128:(g + 1) * 128, :])
                bhn_t.append(bt_g)

            # ---- phase A: hp^T [128h x 4, 64b] (+bh via ones row) ----
            hpT_sb = wpool.tile([128, 4 * P], F32)
            with tc.tile_pool(name="psA", bufs=2, space="PSUM") as psA:
                for hc in range(4):
                    hp_ps = psA.tile([128, P], F32)
                    for kk in range(5):
                        nc.tensor.matmul(
                            hp_ps[:],
                            whE_sb[:, kk * H + hc * 128:kk * H + (hc + 1) * 128],
                            phTe_sb[:, kk * P:(kk + 1) * P],
                            start=(kk == 0), stop=(kk == 4),
                        )
                    nc.vector.tensor_copy(hpT_sb[:, hc * P:(hc + 1) * P], hp_ps[:])

            # ---- phase B: proj -> tanh -> e ----
            # e chunks land at partition base 32*(j%4), column slot j//4
            # (DVE dest partition bases must be 32-aligned)
            e_stage = wpool.tile([128, (NJ // 4) * 512], F32)
            with (
                tc.tile_pool(name="psB", bufs=5, space="PSUM") as psB,
                tc.tile_pool(name="psE", bufs=3, space="PSUM") as psE,
            ):
                # e-matmuls are software-pipelined 2 (j,hc)-groups behind the
                # proj/tanh that feed them, so the in-order PE queue never
                # head-of-line-blocks on an unfinished ACT pair.
                pending = []
                last_proj = [None]

                def flush_e():
                    # emit all 4 e-matmuls of one j back-to-back: the PE pays
                    # its 128<->32 tile-size reconfig penalty twice per j
                    # instead of on every e-matmul
                    for _ in range(4):
                        e_ps_f, hc_f, tanh_f, j_f = pending.pop(0)
                        mm = nc.tensor.matmul(
                            e_ps_f[:],
                            ws_sb[:, hc_f:hc_f + 1],
                            tanh_f[:],
                            start=(hc_f == 0), stop=(hc_f == 3),
                        )
                        if last_proj[0] is not None:
                            add_dep_helper(mm.ins, last_proj[0].ins, False,
                                           "software-pipeline e-mm behind proj")
                    g, s = (j_f % 4) * 32, j_f // 4
                    nc.scalar.copy(
                        e_stage[g:g + 1, s * 512:(s + 1) * 512], e_ps_f[:])

                bht_t = None
                for j in range(NJ):
                    if j % 4 == 0:
                        bht_t = [bht_pool.tile([128, 4 * 512], BF16, tag=f"bht{kc}",
                                               name=f"bht_t{j}_{kc}")
                                 for kc in range(4)]
                        for kc in range(4):
                            if j == 0:
                                # split the first group per-j so early slices
                                # land ASAP and the PE can start early
                                for jj2 in range(4):
                                    nc.sync.dma_start(
                                        bht_t[kc][:, jj2 * 512:(jj2 + 1) * 512],
                                        bhT[kc * 128:(kc + 1) * 128,
                                            jj2 * 512:(jj2 + 1) * 512],
                                    )
                            elif j == 4:
                                for jj2 in range(2):
                                    nc.sync.dma_start(
                                        bht_t[kc][:, jj2 * 1024:(jj2 + 1) * 1024],
                                        bhT[kc * 128:(kc + 1) * 128,
                                            (4 + jj2 * 2) * 512:(6 + jj2 * 2) * 512],
                                    )
                            else:
                                nc.sync.dma_start(
                                    bht_t[kc][:],
                                    bhT[kc * 128:(kc + 1) * 128,
                                        j * 512:(j + 4) * 512],
                                )
                    jj = j % 4
                    e_ps = psE.tile([1, 512], F32)
                    for hc in range(4):
                        proj_ps = psB.tile([128, 512], F32)
                        for kc in range(4):
                            last_proj[0] = nc.tensor.matmul(
                                proj_ps[:],
                                wi_sb[:, kc * H + hc * 128:kc * H + (hc + 1) * 128],
                                bht_t[kc][:, jj * 512:(jj + 1) * 512],
                                start=(kc == 0), stop=(kc == 3),
                            )
                        # hp bias-add on DVE (PSUM -> SBUF f32 staging), then a
                        # single whole-tile tanh on ACT — keeps every engine
                        # under the PE group pace
                        parg_t = parg_pool.tile([128, 512], F32)
                        for half in range(2):
                            b_idx = 2 * j + half
                            nc.vector.tensor_scalar_add(
                                parg_t[:, half * 256:(half + 1) * 256],
                                proj_ps[:, half * 256:(half + 1) * 256],
                                hpT_sb[:, hc * P + b_idx:hc * P + b_idx + 1],
                            )
                        tanh_t = tanh_pool.tile([128, 512], BF16)
                        nc.scalar.activation(tanh_t[:], parg_t[:], AF.Tanh)
                        pending.append((e_ps, hc, tanh_t, j))
                        if len(pending) >= 8:
                            flush_e()
                while pending:
                    flush_e()

            # ---- phase C: softmax over t, alpha block-diagonal build ----
            # DRAM round-trip to reshape [(j) chunks] -> [b, t]; the read DMA
            # gets an explicit dep on the write DMA (DRAM is not Tile-tracked)
            ew = nc.sync.dma_start(
                e_tmp[:].rearrange("(s g n) -> g s n", s=NJ // 4, g=4),
                e_stage[0:128:32, :].rearrange("g (s n) -> g s n", s=NJ // 4),
            )
            e_bt = wpool.tile([P, T], F32)
            er = nc.sync.dma_start(e_bt[:], e_tmp[:].rearrange("(b t) -> b t", b=P))
            add_dep_helper(er.ins, ew.ins, True, "e_tmp write -> read ordering")

            mx_neg = wpool.tile([P, 1], F32)
            nc.vector.tensor_reduce(mx_neg[:], e_bt[:], mybir.AxisListType.X,
                                    ALU.max, negate=True)
            ex = wpool.tile([P, T], F32)
            nc.scalar.activation(ex[:], e_bt[:], AF.Exp, bias=mx_neg[:])
            ssum = wpool.tile([P, 1], F32)
            nc.vector.tensor_reduce(ssum[:], ex[:], mybir.AxisListType.X, ALU.add)
            rsum = wpool.tile([P, 1], F32)
            nc.vector.reciprocal(rsum[:], ssum[:])
            alpha_sb = wpool.tile([P, T], F32)
            nc.vector.tensor_scalar_mul(alpha_sb[:], ex[:], rsum[:])
            nc.sync.dma_start(alpha_out[:], alpha_sb[:])

            with tc.tile_pool(name="psC", bufs=2, space="PSUM") as psC:
                for i in range(2):
                    at_ps = psC.tile([128, P], F32)
                    nc.tensor.transpose(
                        at_ps[:], alpha_sb[0:P, i * 128:(i + 1) * 128],
                        ident_f[0:P, 0:P],
                    )
                    # scatter columns into the block-diagonal layout:
                    # dest free offset = 129*b + 64*i, stride 129, count 64
                    nc.vector.tensor_copy(
                        bd_sb[:, P * i:P * i + 129 * (P - 1) + 1:129], at_ps[:]
                    )

            # ---- phase D: context [64, 512] ----
            ctx_bf = wpool.tile([P, H], BF16)
            with tc.tile_pool(name="psD", bufs=1, space="PSUM") as psD:
                ctx_ps = psD.tile([P, H], F32)
                for kt in range(NKT):
                    g, i = kt // 4, kt % 4
                    nc.tensor.matmul(
                        ctx_ps[:],
                        bd_sb[:, kt * P:(kt + 1) * P],
                        bhn_t[g][:, i * C:(i + 1) * C],
                        start=(kt == 0), stop=(kt == NKT - 1),
                    )
                nc.vector.tensor_copy(ctx_bf[:], ctx_ps[:])

            # ---- phase E: LSTM cell ----
            xstack = wpool.tile([128, 9 * P], BF16)
            nc.sync.dma_start(xstack[:, 4 * P:9 * P], xrt[:])
            with tc.tile_pool(name="psT", bufs=2, space="PSUM") as psT:
                for q in range(4):
                    ct_ps = psT.tile([128, P], BF16)
                    nc.tensor.transpose(
                        ct_ps[:], ctx_bf[0:P, q * 128:(q + 1) * 128],
                        ident_b[0:P, 0:P],
                    )
                    nc.vector.tensor_copy(xstack[:, q * P:(q + 1) * P], ct_ps[:])

            gi = wpool.tile([P, H], F32)
            gf = wpool.tile([P, H], F32)
            gg = wpool.tile([P, H], F32)
            go = wpool.tile([P, H], F32)
            with tc.tile_pool(name="psZ", bufs=1, space="PSUM") as psZ:
                z_ps = psZ.tile([P, 4 * H], F32)
                for kk in range(9):
                    kr_t = kr_pool.tile([128, 4 * H], BF16, tag="kr", name=f"kr_t{kk}")
                    nc.sync.dma_start(kr_t[:], krE[kk * 128:(kk + 1) * 128, :])
                    for nn in range(4):
                        nc.tensor.matmul(
                            z_ps[:, nn * 512:(nn + 1) * 512],
                            xstack[:, kk * P:(kk + 1) * P],
                            kr_t[:, nn * 512:(nn + 1) * 512],
                            start=(kk == 0), stop=(kk == 8),
                        )
                nc.scalar.activation(gi[:], z_ps[:, 0:512], AF.Sigmoid)
                nc.scalar.activation(gf[:], z_ps[:, 512:1024], AF.Sigmoid)
                nc.scalar.activation(gg[:], z_ps[:, 1024:1536], AF.Tanh)
                nc.scalar.activation(go[:], z_ps[:, 1536:2048], AF.Sigmoid)

            pc_sb = wpool.tile([P, H], F32)
            nc.sync.dma_start(pc_sb[:], pc[:])
            t1 = wpool.tile([P, H], F32)
            t2 = wpool.tile([P, H], F32)
            c_sb = wpool.tile([P, H], F32)
            nc.vector.tensor_mul(t1[:], gf[:], pc_sb[:])
            nc.vector.tensor_mul(t2[:], gi[:], gg[:])
            nc.vector.tensor_add(c_sb[:], t1[:], t2[:])
            nc.sync.dma_start(c_out[:], c_sb[:])
            tc_sb = wpool.tile([P, H], F32)
            nc.scalar.activation(tc_sb[:], c_sb[:], AF.Tanh)
            h_sb = wpool.tile([P, H], F32)
            nc.vector.tensor_mul(h_sb[:], go[:], tc_sb[:])
            nc.sync.dma_start(h_out[:], h_sb[:])

    nc.compile()
    return nc


def _get_nc():
    global _CACHED
    if _CACHED is None:
        _CACHED = _build()
    return _CACHED


def _prep_in_maps(inputs):
    batch_H = np.asarray(inputs["batch_H"], np.float32)
    prev_h = np.asarray(inputs["prev_h"], np.float32)
    prev_c = np.asarray(inputs["prev_c"], np.float32)
    onehots = np.asarray(inputs["char_onehots"], np.float32)
    Wi = np.asarray(inputs["Wi"], np.float32)
    Wh = np.asarray(inputs["Wh"], np.float32)
    bh = np.asarray(inputs["bh"], np.float32)
    Ws = np.asarray(inputs["Ws"], np.float32)
    K = np.asarray(inputs["K"], np.float32)
    R = np.asarray(inputs["R"], np.float32)
    b = np.asarray(inputs["b"], np.float32)

    def pack(a, k):  # [(k p), n] -> [p, (k n)]
        n = a.shape[1]
        return np.ascontiguousarray(
            a.reshape(k, 128, n).transpose(1, 0, 2).reshape(128, k * n))

    wi_b = pack(Wi, 4).astype(bf16)
    ws_b = np.ascontiguousarray(Ws.reshape(4, 128).T).astype(bf16)
    whE = np.zeros((640, H), np.float32)
    whE[:H] = Wh
    whE[H] = bh
    whE_b = pack(whE, 5).astype(bf16)
    krE = np.zeros((KXP, 4 * H), np.float32)
    krE[:C + NCLS] = K
    krE[C + NCLS:C + NCLS + H] = R
    krE[C + NCLS + H] = b
    krE_b = krE.astype(bf16)

    in_maps = []
    for i in range(NCORES):
        sl = slice(i * P, (i + 1) * P)
        bh_s = batch_H[sl].reshape(BT, C)
        ph_s = prev_h[sl]                       # [64, 512]
        phTe = np.zeros((640, P), np.float32)
        phTe[:H] = ph_s.T
        phTe[H] = 1.0
        xrt = np.zeros((640, P), np.float32)
        xrt[:NCLS] = onehots[sl].T
        xrt[NCLS:NCLS + H] = ph_s.T
        xrt[NCLS + H] = 1.0
        bhn_p = np.ascontiguousarray(
            bh_s.reshape(32, 4, 128, C).transpose(0, 2, 1, 3).reshape(BT // 4, 4 * C))
        in_maps.append({
            "bhT": np.ascontiguousarray(bh_s.T).astype(bf16),
            "bhN": bhn_p.astype(bf16),
            "wi": wi_b,
            "whE": whE_b,
            "phTe": pack(phTe, 5).astype(bf16),
            "ws": ws_b,
            "xrt": pack(xrt, 5).astype(bf16),
            "krE": krE_b,
            "pc": prev_c[sl],
        })
    return in_maps


def run(inputs, **run_kwargs):
    nc = _get_nc()
    in_maps = _prep_in_maps(inputs)
    res = run_bass_kernel_spmd(nc, in_maps, core_ids=list(range(NCORES)),
                               **run_kwargs)
    h = np.concatenate([res.results[i]["h_out"] for i in range(NCORES)], 0)
    c = np.concatenate([res.results[i]["c_out"] for i in range(NCORES)], 0)
    alpha = np.concatenate([res.results[i]["alpha_out"] for i in range(NCORES)], 0)
    return (h, c, alpha.reshape(B, T, 1)), res


def kernel(**inputs):
    (h, c, alpha), _ = run(inputs)
    return (h, c, alpha)
